# revision 1
# baseline (speedup 1.0000x reference)
"""ACDC layer (A*x -> DCT -> D* -> riffle -> IDCT -> +bias) on 8 TRN2 NeuronCores.

The whole per-group chain is a fixed linear map: for each group g (4 groups of
1024 columns), out[:, g] = x[:, g] @ M_g^T + bias_g with
M_g = Cinv @ P_riffle @ diag(D_g) @ C @ diag(A_g) precomputed on host.

Kernel: pure data parallel over rows (2048 rows/core). Per 128-row tile:
  - SWDGE cast-DMA loads x tile f32 -> bf16
  - PE transposes the tile (bf16) into PSUM, ACT copies PSUM->SBUF
  - 4 groups x 2 dout-chunks x 8 k-chunk accumulating matmuls (bf16)
  - DVE adds bias while copying PSUM->SBUF, HWDGE stores f32
"""

import numpy as np
import ml_dtypes

import concourse.bass as bass
import concourse.tile as tile
from concourse import bacc, mybir
from concourse.bass_utils import run_bass_kernel_spmd
from concourse.masks import make_identity

N_CORES = 8
N_FULL, D = 16384, 4096
GROUPS = 4
DG = D // GROUPS          # 1024
ROWS_PER_CORE = N_FULL // N_CORES  # 2048
P = 128
N_TILES = ROWS_PER_CORE // P       # 16
K_CHUNKS = DG // P                 # 8
NW = GROUPS * K_CHUNKS             # 32 weight chunks of [128, 1024]
DOUT_SPLIT = 512
DOUT_CHUNKS = DG // DOUT_SPLIT     # 2

_CACHE: dict = {}
LAST_RESULT = None  # BassKernelResults of the most recent run (for test harness)


def _ensure_profile_hook_module():
    """If BASS_TRACE is set but the image's antenv lacks axon_hooks,
    run_bass_kernel_spmd would crash on import; provide a no-op module so it
    degrades to 'no trace captured' instead."""
    try:
        import antenv.axon_hooks  # noqa: F401
    except Exception:
        try:
            import sys
            import types

            import antenv

            m = types.ModuleType("antenv.axon_hooks")
            m._h = None
            m.get_axon_ntff_profile_hook = lambda: m._h
            m.set_axon_ntff_profile_hook = lambda h: setattr(m, "_h", h)
            sys.modules["antenv.axon_hooks"] = m
            antenv.axon_hooks = m
        except Exception:
            pass


def _build_weights(A: np.ndarray, Dv: np.ndarray) -> np.ndarray:
    """Fused per-group matrices as bf16, layout [NW, 128, 1024] where
    W[g*K_CHUNKS + kc] = M_g^T[kc*128:(kc+1)*128, :]."""
    N = DG
    j = np.arange(N)[None, :]
    k = np.arange(N)[:, None]
    C = 2.0 * np.cos(np.pi * (j + 0.5) * k / N)  # dct(v) = C @ v
    kk = np.arange(N)[None, :]
    jj = np.arange(N)[:, None]
    w0 = np.ones(N)
    w0[0] = 0.5
    Cinv = (1.0 / N) * w0[None, :] * np.cos(np.pi * kk * (jj + 0.5) / N)
    perm = np.arange(N).reshape(N // 2, 2).T.reshape(N)

    W = np.empty((NW, P, DG), dtype=ml_dtypes.bfloat16)
    for g in range(GROUPS):
        Ag = A[0, g * DG:(g + 1) * DG].astype(np.float64)
        Dg = Dv[0, g * DG:(g + 1) * DG].astype(np.float64)
        M = Cinv @ ((Dg[:, None] * C * Ag[None, :])[perm])  # [dout, din]
        MT = np.ascontiguousarray(M.T)  # [din, dout]
        for kc in range(K_CHUNKS):
            W[g * K_CHUNKS + kc] = MT[kc * P:(kc + 1) * P, :].astype(
                ml_dtypes.bfloat16
            )
    return W


def _build_kernel():
    nc = bacc.Bacc("TRN2", target_bir_lowering=False, debug=False)

    x_ext = nc.declare_dram_parameter(
        "x", [ROWS_PER_CORE, D], mybir.dt.float32, isOutput=False
    )
    w_ext = nc.declare_dram_parameter(
        "w", [NW, P, DG], mybir.dt.bfloat16, isOutput=False
    )
    b_ext = nc.declare_dram_parameter(
        "bias", [P, D], mybir.dt.float32, isOutput=False
    )
    out_ext = nc.declare_dram_parameter(
        "out", [ROWS_PER_CORE, D], mybir.dt.float32, isOutput=True
    )

    with tile.TileContext(nc) as tc:
        with (
            tc.tile_pool(name="consts", bufs=1) as consts,
            tc.tile_pool(name="xin", bufs=4) as xin_pool,
            tc.tile_pool(name="xt", bufs=3) as xt_pool,
            tc.tile_pool(name="outp", bufs=3) as out_pool,
            tc.tile_pool(name="pt", bufs=3, space=bass.MemorySpace.PSUM) as pt_pool,
            tc.tile_pool(name="pacc", bufs=4, space=bass.MemorySpace.PSUM) as pa_pool,
        ):
            ident = consts.tile([P, P], mybir.dt.bfloat16)
            make_identity(nc, ident)

            w_sb = consts.tile([P, NW, DG], mybir.dt.bfloat16)
            for c in range(NW):
                nc.sync.dma_start(out=w_sb[:, c, :], in_=w_ext[c])

            bias_sb = consts.tile([P, D], mybir.dt.float32)
            nc.scalar.dma_start(out=bias_sb, in_=b_ext[:])

            for t in range(N_TILES):
                # load + cast f32 -> bf16 in flight (SWDGE)
                xb = xin_pool.tile([P, D], mybir.dt.bfloat16)
                nc.gpsimd.dma_start(out=xb, in_=x_ext[t * P:(t + 1) * P, :])

                out_sb = out_pool.tile([P, D], mybir.dt.float32)
                # group-granular: transpose 8 chunks on PE, drain to SBUF on
                # ACT/DVE alternately, then that group's 16 matmuls — PE keeps
                # a continuous T(g+1)/mm(g) stream, copies hide under matmuls
                for g in range(GROUPS):
                    pt = pt_pool.tile([P, DG], mybir.dt.bfloat16)
                    for kc in range(K_CHUNKS):
                        c = g * K_CHUNKS + kc
                        nc.tensor.transpose(
                            pt[:, kc * P:(kc + 1) * P],
                            xb[:, c * P:(c + 1) * P],
                            ident,
                        )
                    xtg = xt_pool.tile([P, DG], mybir.dt.bfloat16)
                    if g % 2 == 0:
                        nc.scalar.copy(out=xtg, in_=pt)
                    else:
                        nc.vector.tensor_copy(xtg, pt)

                    for dc in range(DOUT_CHUNKS):
                        acc = pa_pool.tile([P, DOUT_SPLIT], mybir.dt.float32)
                        for kc in range(K_CHUNKS):
                            ci = g * K_CHUNKS + kc
                            nc.tensor.matmul(
                                acc,
                                xtg[:, kc * P:(kc + 1) * P],
                                w_sb[:, ci, dc * DOUT_SPLIT:(dc + 1) * DOUT_SPLIT],
                                start=(kc == 0),
                                stop=(kc == K_CHUNKS - 1),
                            )
                        col0 = g * DG + dc * DOUT_SPLIT
                        nc.vector.tensor_add(
                            out_sb[:, col0:col0 + DOUT_SPLIT],
                            acc,
                            bias_sb[:, col0:col0 + DOUT_SPLIT],
                        )

                nc.sync.dma_start(
                    out=out_ext[t * P:(t + 1) * P, :], in_=out_sb
                )

    nc.compile()
    return nc


def kernel(x, A, D, bias):
    global LAST_RESULT
    x = np.ascontiguousarray(np.asarray(x, dtype=np.float32))
    A = np.asarray(A, dtype=np.float32)
    Dv = np.asarray(D, dtype=np.float32)
    bias = np.ascontiguousarray(
        np.broadcast_to(
            np.asarray(bias, dtype=np.float32).reshape(1, 4096), (P, 4096)
        )
    )

    W = _build_weights(A, Dv)

    _ensure_profile_hook_module()
    if "nc" not in _CACHE:
        _CACHE["nc"] = _build_kernel()
    nc = _CACHE["nc"]

    in_maps = [
        {
            "x": x[i * ROWS_PER_CORE:(i + 1) * ROWS_PER_CORE],
            "w": W,
            "bias": bias,
        }
        for i in range(N_CORES)
    ]
    res = run_bass_kernel_spmd(nc, in_maps, core_ids=list(range(N_CORES)))
    LAST_RESULT = res
    out = np.concatenate(
        [res.results[i]["out"] for i in range(N_CORES)], axis=0
    )
    return out

